# revision 3
# baseline (speedup 1.0000x reference)
"""RBF Gram kernel + symmetric degree normalization on 8 Trainium2 cores.

out = D^-1/2 K D^-1/2,  K_ij = exp(-||x_i - x_j||^2 / (2 sigma^2)),
x: [8192, 128] fp32.

Strategy (row-sharded, two-pass recompute):
  - Host: yT = (x / (sigma*sqrt(2))).T as fp16 [128, BS]; t_j = |y_j|^2 (fp32).
  - exp factorization: K_ij * ds_i * ds_j = exp(2*(y_i.y_j + c_j) + b_i) with
    the per-column term c_j folded into the matmul as 2 extra fp16 contraction
    rows (hi/lo split for fp32-grade accuracy) and the per-row term b_i applied
    via the ACT engine's per-partition bias. No per-element DVE work at all.
  - Pass 1: each core computes row sums of K over its 1024 rows using
    activation(Exp, accum_out=...) directly from PSUM -> [1024] local sums.
  - AllGather (32 KB) -> full degree vector; aug2/bias2 computed on device.
  - Pass 2: recompute tiles with the normalization folded in, DMA out.
"""

import numpy as np

import concourse.bass as bass
import concourse.bacc as bacc
import concourse.tile as tile
from concourse import mybir
from concourse.bass_interp import get_hw_module
from concourse.bass_utils import run_bass_kernel_spmd

BS = 8192
D = 128
SIGMA = 16.0
CORES = 8
ALPHA = 1.0 / (2.0 * SIGMA * SIGMA)

ROWS = BS // CORES          # rows per core
NB = ROWS // 128            # 128-row blocks per core
NJ = BS // 512              # 512-col matmul tiles per row
GCOLS = 2048                # columns per PSUM group (4 banks)
NG = BS // GCOLS            # groups per row block
TPG = GCOLS // 512          # 512-tiles per group
PCH = BS // 128             # free size of [128, PCH] vector layout

f32 = mybir.dt.float32
f16 = mybir.dt.float16
AF = mybir.ActivationFunctionType


def _emit(ctx, tc, io):
    nc = tc.nc
    yt, ytrows, aug1, ones2, bias1, tm64, out = (
        io["yt"], io["ytrows"], io["aug1"], io["ones2"], io["bias1"],
        io["tm64"], io["out"],
    )

    const = ctx.enter_context(tc.tile_pool(name="const", bufs=1))
    psum = ctx.enter_context(tc.tile_pool(name="psum", bufs=2, space="PSUM"))
    scr = ctx.enter_context(tc.tile_pool(name="scr", bufs=4))
    dram = ctx.enter_context(tc.tile_pool(name="dram", bufs=1, space="DRAM"))

    # resident SBUF data
    yt_sb = const.tile([128, BS], f16)
    nc.sync.dma_start(yt_sb[:], yt[:])
    ytr_sb = const.tile([128, ROWS], f16)
    nc.sync.dma_start(ytr_sb[:], ytrows[:])
    aug1_sb = const.tile([2, BS], f16)
    nc.sync.dma_start(aug1_sb[:], aug1[:])
    ones2_sb = const.tile([2, 128], f16)
    nc.sync.dma_start(ones2_sb[:], ones2[:])
    bias1_sb = const.tile([128, NB], f32)
    nc.sync.dma_start(bias1_sb[:], bias1[:])
    tm64_sb = const.tile([128, PCH], f32)
    nc.sync.dma_start(tm64_sb[:], tm64[:])

    stats_sb = const.tile([128, NB * NG], f32)
    aug2_sb = const.tile([2, BS], f16)
    bias2_sb = const.tile([128, NB], f32)

    def block_pass(b, aug_sb, bias_sb, pass2):
        for g in range(NG):
            pt = psum.tile([128, GCOLS], f32, tag="pt")
            for t in range(TPG):
                j0 = g * GCOLS + t * 512
                sl = pt[:, t * 512:(t + 1) * 512]
                nc.tensor.matmul(sl, ytr_sb[:, b * 128:(b + 1) * 128],
                                 yt_sb[:, j0:j0 + 512], start=True, stop=False)
                nc.tensor.matmul(sl, ones2_sb[:], aug_sb[:, j0:j0 + 512],
                                 start=False, stop=True)
            ot = scr.tile([128, GCOLS], f32, tag="ot")
            if pass2:
                nc.scalar.activation(ot[:], pt[:], AF.Exp,
                                     bias=bias_sb[:, b:b + 1], scale=2.0)
                nc.sync.dma_start(
                    out[b * 128:(b + 1) * 128, g * GCOLS:(g + 1) * GCOLS], ot[:])
            else:
                nc.scalar.activation(ot[:], pt[:], AF.Exp,
                                     bias=bias_sb[:, b:b + 1], scale=2.0,
                                     accum_out=stats_sb[:, b * NG + g:b * NG + g + 1])

    # ---- pass 1: local row sums ----
    for b in range(NB):
        block_pass(b, aug1_sb, bias1_sb, pass2=False)

    # reduce per-group partials -> s_loc [128, NB]
    s_loc = const.tile([128, NB], f32)
    nc.vector.tensor_reduce(
        s_loc[:], stats_sb[:].rearrange("p (b g) -> p b g", g=NG),
        axis=mybir.AxisListType.X, op=mybir.AluOpType.add)

    # ---- AllGather of row sums ----
    cc_in = dram.tile([ROWS], f32)
    cc_out = dram.tile([BS], f32, addr_space="Shared")
    nc.sync.dma_start(cc_in[:].rearrange("(b p) -> p b", p=128), s_loc[:])
    nc.gpsimd.collective_compute(
        "AllGather", mybir.AluOpType.bypass,
        replica_groups=[list(range(CORES))],
        ins=[cc_in.opt()], outs=[cc_out.opt()])

    # ---- bias2 = bias1 - 0.5*ln(s_loc) ----
    lnl = const.tile([128, NB], f32)
    nc.scalar.activation(lnl[:], s_loc[:], AF.Ln)
    nc.vector.tensor_scalar_mul(lnl[:], lnl[:], -0.5)
    nc.vector.tensor_add(bias2_sb[:], lnl[:], bias1_sb[:])

    # ---- aug2 = hi/lo fp16 of (-0.5*t_j - 0.25*ln(s_j)) ----
    s64 = const.tile([128, PCH], f32)
    nc.sync.dma_start(s64[:], cc_out[:].rearrange("(p c) -> p c", p=128))
    c2 = const.tile([128, PCH], f32)
    nc.scalar.activation(c2[:], s64[:], AF.Ln)
    nc.vector.tensor_scalar_mul(c2[:], c2[:], -0.25)
    nc.vector.tensor_add(c2[:], c2[:], tm64_sb[:])
    c2h = const.tile([128, PCH], f16)
    nc.vector.tensor_copy(c2h[:], c2[:])
    c2h32 = const.tile([128, PCH], f32)
    nc.vector.tensor_copy(c2h32[:], c2h[:])
    c2l = const.tile([128, PCH], f32)
    nc.vector.tensor_sub(c2l[:], c2[:], c2h32[:])
    c2l16 = const.tile([128, PCH], f16)
    nc.vector.tensor_copy(c2l16[:], c2l[:])
    hb = dram.tile([BS], f16)
    lb = dram.tile([BS], f16)
    nc.sync.dma_start(hb[:].rearrange("(p c) -> p c", p=128), c2h[:])
    nc.sync.dma_start(lb[:].rearrange("(p c) -> p c", p=128), c2l16[:])
    nc.sync.dma_start(aug2_sb[0:1, :], hb[:].rearrange("(a j) -> a j", a=1))
    nc.sync.dma_start(aug2_sb[1:2, :], lb[:].rearrange("(a j) -> a j", a=1))

    # ---- pass 2: normalized tiles -> out ----
    for b in range(NB):
        block_pass(b, aug2_sb, bias2_sb, pass2=True)


_CACHE = {}


def _build():
    if "nc" in _CACHE:
        return _CACHE["nc"]
    nc = bacc.Bacc("TRN2", target_bir_lowering=False, debug=False,
                   enable_asserts=False, num_devices=CORES)
    io = {
        "yt": nc.dram_tensor("yt", [128, BS], f16, kind="ExternalInput").ap(),
        "ytrows": nc.dram_tensor("ytrows", [128, ROWS], f16,
                                 kind="ExternalInput").ap(),
        "aug1": nc.dram_tensor("aug1", [2, BS], f16, kind="ExternalInput").ap(),
        "ones2": nc.dram_tensor("ones2", [2, 128], f16,
                                kind="ExternalInput").ap(),
        "bias1": nc.dram_tensor("bias1", [128, NB], f32,
                                kind="ExternalInput").ap(),
        "tm64": nc.dram_tensor("tm64", [128, PCH], f32,
                               kind="ExternalInput").ap(),
        "out": nc.dram_tensor("out", [ROWS, BS], f32,
                              kind="ExternalOutput").ap(),
    }
    from contextlib import ExitStack
    with tile.TileContext(nc, trace_sim=False) as tc, ExitStack() as ctx:
        _emit(ctx, tc, io)
    nc.compile()
    nc.m = get_hw_module(nc.m)
    _CACHE["nc"] = nc
    return nc


def _prep_inputs(x):
    x = np.asarray(x, dtype=np.float32)
    y = (x.T * np.float32(np.sqrt(ALPHA))).astype(np.float32)  # [128, BS]
    yt16 = y.astype(np.float16)
    t = (ALPHA * np.sum(x.astype(np.float64) ** 2, axis=1)).astype(np.float32)

    c1 = (-0.5 * t).astype(np.float32)
    c1_hi = c1.astype(np.float16)
    c1_lo = (c1 - c1_hi.astype(np.float32)).astype(np.float16)
    aug1 = np.stack([c1_hi, c1_lo])                      # [2, BS] f16
    ones2 = np.ones((2, 128), dtype=np.float16)
    tm64 = (-0.5 * t).reshape(128, PCH).copy()           # t[p*PCH + c]

    in_maps = []
    for c in range(CORES):
        off = c * ROWS
        rows = slice(off, off + ROWS)
        bias1 = (-t[rows]).reshape(NB, 128).T.copy()     # [128, NB]
        in_maps.append({
            "yt": yt16,
            "ytrows": np.ascontiguousarray(yt16[:, rows]),
            "aug1": aug1,
            "ones2": ones2,
            "bias1": bias1,
            "tm64": tm64,
        })
    return in_maps


def run(x, trace=False):
    nc = _build()
    in_maps = _prep_inputs(x)
    res = run_bass_kernel_spmd(nc, in_maps, core_ids=list(range(CORES)),
                               trace=trace)
    full = np.concatenate([res.results[c]["out"] for c in range(CORES)], axis=0)
    return full, res


def kernel(x):
    full, _ = run(x, trace=False)
    return full
